# revision 1
# baseline (speedup 1.0000x reference)
"""Scaled dot-product attention on 8 Trainium2 NeuronCores.

Problem: B=2, H=16, S=2048, D=128, fp32, mask all-ones.
Sharding: the 32 (b,h) pairs are split 4-per-core across 8 cores; attention is
fully independent per (b,h) so there is no cross-core communication.

Device algorithm (per core, per (b,h)):
  Layouts are chosen so NO on-chip transposes are needed:
    - host feeds Qt, Kt pre-transposed as [D, S]; V natural [S, D]
    - scores are computed transposed: St[k, q] = Kt_chunk.T @ Qt  (contract d)
    - E = exp(scale * St) on ScalarE (PSUM -> SBUF), f32r-rounded
    - out^T[d, q] += V_chunk.T... i.e. matmul(lhsT=V_chunk[k,d], rhs=E[k,q])
      accumulated over the 16 k-chunks in PSUM (contract k)
    - rowsum[q] broadcast to all partitions via matmul(lhsT=all-ones, rhs=E)
    - out^T_norm = out^T * reciprocal(rowsum_bcast) on VectorE
  Host transposes out^T back to [S, D].

Matmuls run in float32r (TF32-like, full PE rate); accumulation is fp32.
"""
import math
import sys

import numpy as np

sys.path.insert(0, "/opt/trn_rl_repo")

B, H, S, D = 2, 16, 2048, 128
N_CORES = 8
BH = B * H
BH_PER_CORE = BH // N_CORES          # 4
SCALE = 1.0 / math.sqrt(D)
QB = 512                              # q-block (one PSUM bank of fp32)
N_QB = S // QB                        # 4
KC = S // 128                         # 16 k-chunks of 128
ACT_PAIR = 2                          # k-chunks exp'd per ACT instruction

_cache = {}


def _build():
    import concourse.bass as bass
    import concourse.tile as tile
    from concourse import bacc, mybir

    f32 = mybir.dt.float32
    f32r = mybir.dt.float32r
    EXP = mybir.ActivationFunctionType.Exp

    nc = bacc.Bacc("TRN2", target_bir_lowering=False, num_devices=N_CORES)
    qt_d = nc.declare_dram_parameter("qt", [BH_PER_CORE, D, S], f32, isOutput=False)
    kt_d = nc.declare_dram_parameter("kt", [BH_PER_CORE, D, S], f32, isOutput=False)
    v_d = nc.declare_dram_parameter("v", [BH_PER_CORE, S, D], f32, isOutput=False)
    ot_d = nc.declare_dram_parameter("ot", [BH_PER_CORE, D, S], f32, isOutput=True)

    with tile.TileContext(nc) as tc:
        with (
            tc.tile_pool(name="const", bufs=1) as constp,
            tc.tile_pool(name="qkv", bufs=2) as qkvp,
            tc.tile_pool(name="e", bufs=3) as ep,
            tc.tile_pool(name="fin", bufs=2) as finp,
            tc.tile_pool(name="st", bufs=2, space="PSUM") as stp,
            tc.tile_pool(name="acc", bufs=1, space="PSUM") as accp,
            tc.tile_pool(name="rs", bufs=1, space="PSUM") as rsp,
        ):
            ones0 = constp.tile([128, 128], f32)
            nc.vector.memset(ones0[:], 1.0)
            ones = constp.tile([128, 128], f32)
            nc.vector.tensor_copy(ones[:].bitcast(f32r), ones0[:])

            for bh in range(BH_PER_CORE):
                qt = qkvp.tile([D, S], f32, tag="qt")
                kt = qkvp.tile([D, S], f32, tag="kt")
                v = qkvp.tile([128, KC, D], f32, tag="v")
                nc.sync.dma_start(qt[:].bitcast(f32r), qt_d[bh].bitcast(f32r))
                nc.sync.dma_start(kt[:].bitcast(f32r), kt_d[bh].bitcast(f32r))
                nc.sync.dma_start(
                    v[:].bitcast(f32r),
                    v_d[bh].rearrange("(a b) d -> b a d", b=128).bitcast(f32r),
                )

                for qb in range(N_QB):
                    acc = accp.tile([128, QB], f32)
                    rs = rsp.tile([128, QB], f32)
                    for kcp in range(KC // ACT_PAIR):
                        st = stp.tile([128, ACT_PAIR * QB], f32)
                        for j in range(ACT_PAIR):
                            kc = kcp * ACT_PAIR + j
                            nc.tensor.matmul(
                                st[:, j * QB:(j + 1) * QB],
                                kt[:, kc * 128:(kc + 1) * 128].bitcast(f32r),
                                qt[:, qb * QB:(qb + 1) * QB].bitcast(f32r),
                                start=True,
                                stop=True,
                            )
                        e = ep.tile([128, ACT_PAIR * QB], f32, tag="e")
                        nc.scalar.activation(e[:].bitcast(f32r), st[:], EXP, scale=SCALE)
                        for j in range(ACT_PAIR):
                            kc = kcp * ACT_PAIR + j
                            echunk = e[:, j * QB:(j + 1) * QB].bitcast(f32r)
                            nc.tensor.matmul(
                                acc[:],
                                v[:, kc, :].bitcast(f32r),
                                echunk,
                                start=(kc == 0),
                                stop=(kc == KC - 1),
                            )
                            nc.tensor.matmul(
                                rs[:],
                                ones[:].bitcast(f32r),
                                echunk,
                                start=(kc == 0),
                                stop=(kc == KC - 1),
                            )
                    recip = finp.tile([128, QB], f32, tag="recip")
                    scratch = finp.tile([128, QB], f32, tag="scratch")
                    nc.vector.reciprocal_approx_accurate(recip[:], rs[:], scratch[:])
                    outn = finp.tile([128, QB], f32, tag="outn")
                    nc.vector.tensor_mul(outn[:], acc[:], recip[:])
                    nc.sync.dma_start(ot_d[bh, :, qb * QB:(qb + 1) * QB], outn[:])

    nc.compile()
    return nc


def kernel(query, key, value, mask=None):
    from concourse.bass_utils import run_bass_kernel_spmd

    q = np.ascontiguousarray(np.asarray(query, dtype=np.float32)).reshape(BH, S, D)
    k = np.ascontiguousarray(np.asarray(key, dtype=np.float32)).reshape(BH, S, D)
    v = np.ascontiguousarray(np.asarray(value, dtype=np.float32)).reshape(BH, S, D)

    if "nc" not in _cache:
        _cache["nc"] = _build()
    nc = _cache["nc"]

    in_maps = []
    for c in range(N_CORES):
        sl = slice(c * BH_PER_CORE, (c + 1) * BH_PER_CORE)
        in_maps.append({
            "qt": np.ascontiguousarray(q[sl].transpose(0, 2, 1)),
            "kt": np.ascontiguousarray(k[sl].transpose(0, 2, 1)),
            "v": np.ascontiguousarray(v[sl]),
        })

    res = run_bass_kernel_spmd(nc, in_maps, core_ids=list(range(N_CORES))).results
    out = np.concatenate(
        [np.asarray(r["ot"]).transpose(0, 2, 1) for r in res], axis=0
    )
    return np.ascontiguousarray(out.reshape(B, H, S, D)).astype(np.float32)
